# revision 9
# baseline (speedup 1.0000x reference)
"""GQA causal-attention prefill (B=2, T=S=2048, D=2048, N=16, K=4, H=128)
on 8 Trainium2 NeuronCores.

Sharding: one (batch, kv-head) pair per core -> 2*4 = 8 cores, zero
cross-core communication on device; the o_proj partial sums (over each
batch's 4 kv-head groups) are reduced on the host at unshard time.

Per-core dataflow (layouts avoid on-device transposes of big operands;
host pre-transposes Xq/Xkv once and ships X/W/cos/sin as float16 —
halves HBM traffic; values are small so fp16 keeps ~0.05% precision,
vs bf16's 0.4%):
  QT_n[h,t] = Wq_n^T X^T     (lhsT=Wq f16, rhs=XqT f16)  + RoPE -> f16
  KT[h,s]   = Wk^T Xkv^T                                 + RoPE -> f16
  VT[h,s]   = Wv^T Xkv^T  -> V[s,h] via PE transpose (stored bf16)
  scoresT[s,t] = KT_blk^T QT  (f16xf16 -> f32 PSUM)
  probsT = exp(scoresT + causal mask)    (ACT, unnormalized, bf16 for
                                          range: scores reach ~±50)
  OT_n[h,t] += V_blk^T probsT ;  den[1,t] += ones^T probsT
  inv = 1/den (DVE) -> partition_broadcast (GPSIMD) -> OT_n *= inv
  O[t,d]  = sum_n OT_n^T Wo_n  (f16 x f16 -> f32 PSUM)

Scheduling notes:
 - one PSUM tag set shared across all phases; no pool-close barriers.
 - o_proj of chunk c-1 is emitted quarter-by-quarter after each head of
   chunk c, so the PE never waits on softmax-normalization chains and
   the PSUM->SBUF output copies spread across DVE/ACT without
   head-of-line-blocking the attention-critical ops.
 - inv broadcast on GPSIMD (DMA), not the PE.
 - phase-1 DMAs are batched and issued round-robin across the
   SP/GPSIMD/ACT queues; the first K-proj inputs go first.
 - PE warmup runs off a DVE memset tile (no DMA dependency).
"""
import sys
import types

import numpy as np

try:  # make trace=True degrade gracefully when axon_hooks is absent
    import antenv.axon_hooks  # noqa: F401
except Exception:
    try:
        import antenv
        _m = types.ModuleType("antenv.axon_hooks")
        _h = [None]
        _m.set_axon_ntff_profile_hook = lambda h: _h.__setitem__(0, h)
        _m.get_axon_ntff_profile_hook = lambda: _h[0]
        sys.modules["antenv.axon_hooks"] = _m
        antenv.axon_hooks = _m
    except Exception:
        pass

import concourse.bass as bass
from concourse import bacc
import concourse.tile as tile
import concourse.mybir as mybir
from concourse.bass_utils import run_bass_kernel_spmd
from concourse.masks import make_identity

B, T, D = 2, 2048, 2048
N, K, H = 16, 4, 128
G = N // K
HALF = H // 2
MIN_TS, MAX_TS = 1.0, 10000.0

P = 128
TCH = 512
NCH = T // TCH          # 4 t-chunks of 512
DB = D // P             # 16 contraction blocks
F32 = mybir.dt.float32
F16 = mybir.dt.float16
BF16 = mybir.dt.bfloat16
NEG = -1.0e30
EXP = mybir.ActivationFunctionType.Exp

_CACHE = {}
LAST = None             # BassKernelResults of the most recent run


def _rope_from_psum(nc, pool, pspool, ps, dst, cos_ap, sin_ap, pi_sb):
    """dst[128,TCH](f16) = x*cosF + rot(x)*sinF; rot via PE perm matmul."""
    nc.scalar.copy(dst, ps[:])
    rot_ps = pspool.tile([P, TCH], F32, tag="bb")
    nc.tensor.matmul(rot_ps[:], pi_sb, dst, start=True, stop=True)
    prod = pool.tile([P, TCH], F32, tag="prod")
    nc.vector.tensor_mul(prod[:], rot_ps[:], sin_ap)
    nc.vector.tensor_mul(dst, dst, cos_ap)
    nc.vector.tensor_add(dst, dst, prod[:])


def _build():
    if "nc" in _CACHE:
        return _CACHE["nc"]
    nc = bacc.Bacc(None, target_bir_lowering=False, debug=False)
    xq = nc.declare_dram_parameter("xqT", [D, T], F16, isOutput=False)
    xkv = nc.declare_dram_parameter("xkvT", [D, T], F16, isOutput=False)
    wq = nc.declare_dram_parameter("wq", [D, G * H], F16, isOutput=False)
    wk = nc.declare_dram_parameter("wk", [D, H], F16, isOutput=False)
    wv = nc.declare_dram_parameter("wv", [D, H], F16, isOutput=False)
    wo = nc.declare_dram_parameter("wo", [G, H, D], F16, isOutput=False)
    cq = nc.declare_dram_parameter("cosq", [P, T], F16, isOutput=False)
    sq = nc.declare_dram_parameter("sinq", [P, T], F16, isOutput=False)
    tri = nc.declare_dram_parameter("tri", [P, P], F32, isOutput=False)
    one_c = nc.declare_dram_parameter("one_c", [P, 1], BF16, isOutput=False)
    pi = nc.declare_dram_parameter("pi", [P, P], F16, isOutput=False)
    out = nc.declare_dram_parameter("O", [T, D], F32, isOutput=True)

    xq_v = xq[:].rearrange("(do di) t -> di do t", di=P)
    xkv_v = xkv[:].rearrange("(do di) t -> di do t", di=P)
    wq_v = wq[:].rearrange("(do di) nh -> di do nh", di=P)
    wk_v = wk[:].rearrange("(do di) h -> di do h", di=P)
    wv_v = wv[:].rearrange("(do di) h -> di do h", di=P)
    wo_v = wo[:].rearrange("n h d -> h n d")

    with tile.TileContext(nc) as tc:
        # one PSUM pool, tags shared across phases: never closes, so no
        # phase-boundary drain. 3+2+2+1 = 8 banks.
        with tc.tile_pool(name="glob", bufs=1) as glob, \
             tc.tile_pool(name="psum", bufs=1, space="PSUM") as psum, \
             tc.tile_pool(name="xp", bufs=1) as xp, \
             tc.tile_pool(name="pt", bufs=3) as pt, \
             tc.tile_pool(name="pbp", bufs=10) as pbp, \
             tc.tile_pool(name="otp", bufs=2) as otp, \
             tc.tile_pool(name="smp", bufs=2) as smp, \
             tc.tile_pool(name="obp", bufs=3) as obp:
            qt = glob.tile([P, G, T], F16)
            kt = glob.tile([P, T], F16)
            vsb = glob.tile([P, DB, H], BF16)
            tri_sb = glob.tile([P, P], F32)
            ones_col = glob.tile([P, 1], BF16)
            ident = glob.tile([P, P], F32)
            pi_sb = glob.tile([P, P], F16)
            wq_sb = glob.tile([P, DB, G * H], F16)
            wk_sb = glob.tile([P, DB, H], F16)
            wv_sb = glob.tile([P, DB, H], F16)
            cosq_sb = glob.tile([P, T], F16)
            sinq_sb = glob.tile([P, T], F16)
            wo_sb = glob.tile([P, G, D], F16)
            wsrc = glob.tile([P, P + TCH], BF16)

            # PE warmup gated only on a DVE memset (no DMA).
            nc.vector.memset(wsrc[:], 0.0)
            for i in range(24):
                wtile = psum.tile([P, TCH], F32, tag="aa")
                nc.tensor.matmul(wtile[:], wsrc[:, :P], wsrc[:, P:],
                                 start=True, stop=True)

            dma_engs = [nc.sync, nc.gpsimd, nc.scalar]
            _ei = [0]

            def dma(dst, src):
                dma_engs[_ei[0] % 3].dma_start(dst, src)
                _ei[0] += 1

            xkv_t = {}
            xq_t = {}

            def fetch_kv(c):
                if c >= NCH:
                    return
                tsl = slice(c * TCH, (c + 1) * TCH)
                for i in range(8):
                    xt = xp.tile([P, 2, TCH], F16, tag="xkv", bufs=10)
                    dma(xt[:], xkv_v[:, 2 * i:2 * i + 2, tsl])
                    xkv_t[(c, i)] = xt

            def fetch_q(c):
                if c >= NCH:
                    return
                tsl = slice(c * TCH, (c + 1) * TCH)
                for i in range(8):
                    xt = xp.tile([P, 2, TCH], F16, tag="xq", bufs=10)
                    dma(xt[:], xq_v[:, 2 * i:2 * i + 2, tsl])
                    xq_t[(c, i)] = xt

            nc.gpsimd.dma_start(pi_sb[:], pi[:])
            nc.gpsimd.dma_start(tri_sb[:], tri[:])
            nc.gpsimd.dma_start(ones_col[:], one_c[:])
            make_identity(nc, ident[:])
            dma(wk_sb[:], wk_v)
            fetch_kv(0)
            dma(cosq_sb[:], cq[:])
            dma(sinq_sb[:], sq[:])
            dma(wv_sb[:], wv_v)
            fetch_q(0)
            for i in range(4):
                dma(wq_sb[:, 4 * i:4 * i + 4], wq_v[:, 4 * i:4 * i + 4])
            dma(wo_sb[:, :2], wo_v[:, :2])
            dma(wo_sb[:, 2:], wo_v[:, 2:])

            # ----- Phase 1: per-chunk K/V proj (+RoPE) first, then Q proj
            for c in range(NCH):
                tsl = slice(c * TCH, (c + 1) * TCH)
                ps = psum.tile([P, TCH], F32, tag="aa")
                for db in range(DB):
                    nc.tensor.matmul(ps[:], wk_sb[:, db, :],
                                     xkv_t[(c, db // 2)][:, db % 2, :],
                                     start=(db == 0), stop=(db == DB - 1))
                _rope_from_psum(nc, pt, psum, ps, kt[:, tsl],
                                cosq_sb[:, tsl], sinq_sb[:, tsl], pi_sb[:])
                ps2 = psum.tile([P, TCH], F32, tag="aa")
                for db in range(DB):
                    nc.tensor.matmul(ps2[:], wv_sb[:, db, :],
                                     xkv_t[(c, db // 2)][:, db % 2, :],
                                     start=(db == 0), stop=(db == DB - 1))
                fetch_kv(c + 1)
                vt_tmp = pt.tile([P, TCH], F32, tag="vt")
                nc.scalar.copy(vt_tmp[:], ps2[:])
                for kk in range(4):
                    pst = psum.tile([P, TCH], F32, tag="cc")
                    nc.tensor.transpose(pst[:, :P], vt_tmp[:, kk * P:(kk + 1) * P],
                                        ident[:])
                    nc.scalar.copy(vsb[:, 4 * c + kk, :], pst[:, :P])
                for n in range(G):
                    psq = psum.tile([P, TCH], F32, tag="aa")
                    for db in range(DB):
                        nc.tensor.matmul(
                            psq[:], wq_sb[:, db, n * H:(n + 1) * H],
                            xq_t[(c, db // 2)][:, db % 2, :],
                            start=(db == 0), stop=(db == DB - 1))
                    _rope_from_psum(nc, pt, psum, psq, qt[:, n, tsl],
                                    cosq_sb[:, tsl], sinq_sb[:, tsl],
                                    pi_sb[:])
                    if n == 0:
                        fetch_q(c + 1)

            # ---------- Phase 3: attention + o_proj ----------
            otc_tiles = {}

            def emit_oproj(c, kk):
                # one t-row quarter of chunk c's o_proj; PSUM->SBUF copies
                # alternate DVE/ACT so neither FIFO head-of-line-blocks the
                # attention-critical ops queued behind them.
                otc = otc_tiles[c]
                row = c * TCH + kk * P
                for dc in range(4):
                    ops = psum.tile([P, TCH], F32, tag="cc")
                    for n in range(G):
                        nc.tensor.matmul(
                            ops[:],
                            otc[:, n, kk * P:(kk + 1) * P],
                            wo_sb[:, n, dc * TCH:(dc + 1) * TCH],
                            start=(n == 0), stop=(n == G - 1))
                    osb = obp.tile([P, TCH], F32, tag="osb")
                    if dc % 2 == 0:
                        nc.vector.tensor_copy(osb[:], ops[:])
                    else:
                        nc.scalar.copy(osb[:], ops[:])
                    nc.sync.dma_start(
                        out[row:row + P, dc * TCH:(dc + 1) * TCH],
                        osb[:])

            for c in range(NCH):
                J = 4 * (c + 1)
                otc = otp.tile([P, G, TCH], F16, tag="otc")
                otc_tiles[c] = otc
                for n in range(G):
                    ot_ps = psum.tile([P, TCH], F32, tag="bb")
                    den_ps = psum.tile([1, TCH], F32, tag="dd", bufs=1)
                    for j in range(J):
                        d = j - 4 * c
                        lo = max(d, 0) * P
                        s_ps = psum.tile([P, TCH], F32, tag="aa")
                        nc.tensor.matmul(s_ps[:, lo:],
                                         kt[:, j * P:(j + 1) * P],
                                         qt[:, n, c * TCH + lo:(c + 1) * TCH],
                                         start=True, stop=True)
                        if d >= 0:
                            nc.vector.tensor_add(
                                s_ps[:, d * P:(d + 1) * P],
                                s_ps[:, d * P:(d + 1) * P], tri_sb[:])
                        pb = pbp.tile([P, TCH], BF16, tag="pb")
                        nc.scalar.activation(pb[:, lo:], s_ps[:, lo:], EXP)
                        nc.tensor.matmul(ot_ps[:, lo:], vsb[:, j, :],
                                         pb[:, lo:],
                                         start=(j == 0), stop=(j == J - 1))
                        nc.tensor.matmul(den_ps[:, lo:], ones_col[:],
                                         pb[:, lo:],
                                         start=(j == 0), stop=(j == J - 1))
                    inv_sb = smp.tile([1, TCH], F32, tag="inv_sb")
                    nc.vector.reciprocal_approx_fast(out=inv_sb[:],
                                                     in_=den_ps[:])
                    invb = smp.tile([P, TCH], F32, tag="invb")
                    nc.gpsimd.partition_broadcast(invb[:], inv_sb[:])
                    nc.vector.tensor_mul(otc[:, n, :], ot_ps[:], invb[:])
                    if c > 0:
                        emit_oproj(c - 1, n)
            for kk in range(4):
                emit_oproj(NCH - 1, kk)

    nc.compile()
    _CACHE["nc"] = nc
    return nc


def _rope_tables(pos):
    ts = MIN_TS * (MAX_TS / MIN_TS) ** (2.0 * np.arange(HALF) / H)
    ang = pos.astype(np.float64)[None, :] / ts[:, None]
    c, s = np.cos(ang), np.sin(ang)
    cosF = np.ascontiguousarray(np.concatenate([c, c], 0).astype(np.float16))
    sinF = np.ascontiguousarray(np.concatenate([-s, s], 0).astype(np.float16))
    return cosF, sinF


def kernel(Xq, Xkv, q_positions, kv_positions, Wq, Wk, Wv, Wo, _trace=False):
    global LAST
    from ml_dtypes import bfloat16
    nc = _build()
    Xq = np.asarray(Xq, dtype=np.float32)
    Xkv = np.asarray(Xkv, dtype=np.float32)
    Wq = np.asarray(Wq, dtype=np.float32)
    Wk = np.asarray(Wk, dtype=np.float32)
    Wv = np.asarray(Wv, dtype=np.float32)
    Wo = np.asarray(Wo, dtype=np.float32)
    qp = np.asarray(q_positions)
    kp = np.asarray(kv_positions)
    assert np.array_equal(qp, kp), (
        "kernel assumes q_positions == kv_positions (RoPE tables shared)")

    idx = np.arange(P)
    tri_np = np.where(idx[:, None] <= idx[None, :], 0.0, NEG).astype(np.float32)
    pi_np = np.zeros((P, P), np.float16)
    pi_np[(idx + HALF) % P, idx] = 1.0

    in_maps = []
    for core in range(8):
        b, kv = divmod(core, 4)
        cq_, sq_ = _rope_tables(qp[b])
        in_maps.append({
            "xqT": np.ascontiguousarray(Xq[b].T.astype(np.float16)),
            "xkvT": np.ascontiguousarray(Xkv[b].T.astype(np.float16)),
            "wq": np.ascontiguousarray(
                Wq[:, kv * G:(kv + 1) * G, :].reshape(D, G * H)
            ).astype(np.float16),
            "wk": np.ascontiguousarray(Wk[:, kv, :]).astype(np.float16),
            "wv": np.ascontiguousarray(Wv[:, kv, :]).astype(np.float16),
            "wo": np.ascontiguousarray(
                Wo[kv * G:(kv + 1) * G]).astype(np.float16),
            "cosq": cq_, "sinq": sq_,
            "tri": tri_np,
            "one_c": np.ones((P, 1), bfloat16),
            "pi": pi_np,
        })

    LAST = run_bass_kernel_spmd(nc, in_maps, list(range(8)), trace=_trace)
    parts = [r["O"] for r in LAST.results]
    O = np.stack([parts[0] + parts[1] + parts[2] + parts[3],
                  parts[4] + parts[5] + parts[6] + parts[7]])
    return np.ascontiguousarray(O.astype(np.float32))
